# revision 34
# baseline (speedup 1.0000x reference)
"""Trainium2 Bass kernel for nn_MetaMixer_6717328851330.

Computation (see reference):
    p = x @ W_in.T ; h, gate = split(p) ; gate = silu(gate)
    h = causal_grouped_conv1d(h) + b_conv ; h = h * gate       (residual)
    hn = layernorm_I(h) ; m = silu(hn @ W_fc.T + b_fc) @ W_cp.T + b_cp
    y = (m + residual) @ W_out.T

Strategy: pure data-parallel over the 8192 tokens (B*L), 1024 tokens per
core, no collectives. The causal-conv left halo (3 tokens) is handled by
shipping the previous chunk's last 3 input tokens to each core and running
3-column in_proj matmuls for them.

On-core layout: activations live as [channel, token] tiles so every matmul
in the chain contracts along SBUF partitions with no transposes anywhere
(host pre-transposes x and all weights). All matmul operands are bfloat16
(f32 PSUM accumulate). PSUM limits one accumulation pass to 512 tokens, so
the 1024 tokens run as two segments — but the loop is phase-major: each
streamed weight block serves BOTH segments before being released, so every
weight byte is fetched exactly once (~24MB/core). Biases ride on the
DVE/Act engines (fused scalar_tensor_tensor), not as K=1 matmuls; the
layernorm cross-partition statistics come from DVE running sums over the
res tiles with a single ones-matmul per stat, keeping the PE stream almost
pure GEMM.
"""
import sys

sys.path.insert(0, "/opt/trn_rl_repo")
import numpy as np

NCORES = 8
B, L, H, I, G, CK = 2, 4096, 1024, 2048, 8, 4
T = (B * L) // NCORES          # tokens per core
S = 512                        # token segment (= psum bank free dim)
NSEG = T // S
HK = H // 128                  # 8  k-chunks over hidden
IK = I // 128                  # 16 k-chunks over intermediate
QC = NCORES // B               # seq chunks per batch
EPS = 1e-5
WBUFS = 4                      # streamed weight blocks [128,4096] bf16

_CACHE = {}


def _build():
    import concourse.bacc as bacc
    import concourse.mybir as mybir
    import concourse.tile as tile
    from concourse.alu_op_type import AluOpType

    f32 = mybir.dt.float32
    bf16 = mybir.dt.bfloat16
    AF = mybir.ActivationFunctionType
    MUL, ADD, SUB = AluOpType.mult, AluOpType.add, AluOpType.subtract

    nc = bacc.Bacc(None, target_bir_lowering=False)

    # x ships as 4 pair-tiles [128, 2*(T+4)] so DMA rows are 4KB (queues are
    # packet-rate limited: 2KB rows ~145GB/s, 4KB rows ~183GB/s per queue)
    xT = nc.dram_tensor("xT", [128, 8 * (T + 4)], bf16, kind="ExternalInput")
    win = nc.dram_tensor("win", [128, HK * 2 * I], bf16, kind="ExternalInput")
    cw = nc.dram_tensor("cw", [128, G * 2 * CK * 256], bf16, kind="ExternalInput")
    fcw = nc.dram_tensor("fcw", [128, I * H // 128], bf16, kind="ExternalInput")
    cpw = nc.dram_tensor("cpw", [128, H * I // 128], bf16, kind="ExternalInput")
    outw = nc.dram_tensor("outw", [128, I * H // 128], bf16, kind="ExternalInput")
    # consts consolidated into two tensors (few wide DMAs, not many narrow
    # ones: DMA queues are packet-rate limited, ~11ns per row-packet)
    cstf = nc.dram_tensor("cstf", [128, 48], f32, kind="ExternalInput")
    cstb = nc.dram_tensor("cstb", [128, 192], bf16, kind="ExternalInput")
    yT = nc.dram_tensor("yT", [H, T], bf16, kind="ExternalOutput")

    with nc.allow_low_precision(reason="bf16 matmul pipeline"), \
         tile.TileContext(nc) as tc, \
         tc.tile_pool(name="sb", bufs=1) as sb, \
         tc.tile_pool(name="ps", bufs=1, space="PSUM") as ps:

        def mm_ps():
            return ps.tile([128, S], f32, tag="mm", bufs=7, name="mmps")

        def wblock(src, b):
            """One [128, 4096] weight block = 8 lhsT chunks, single 1MB DMA."""
            w = sb.tile([128, 4096], bf16, tag="wbig", bufs=WBUFS, name="wbig")
            nc.sync.dma_start(w[:], src[:, b * 4096:(b + 1) * 4096])
            return w

        # ---- PE warm-up: dummy matmuls on a zeroed tile (no DMA dependency)
        # spend the cold HAM/p-state window during the ~10us DMA spin-up, so
        # the first real matmuls run at full clock instead of ~600ns
        warm = sb.tile([128, 128], bf16, tag="warm", name="warm")
        nc.vector.memset(warm[:], 0.0)
        pw = ps.tile([128, 128], f32, tag="aux", bufs=1, name="auxps")
        for _ in range(32):
            nc.tensor.matmul(pw[:], warm[:], warm[:], start=True, stop=True)

        # ---- inputs (x data starts at col 4: 8B-aligned rhs), split across
        # two DMA rings so the in_proj feed isn't single-queue limited
        W2 = 2 * (T + 4)
        xkt = []
        for q in range(4):
            t = sb.tile([128, W2], bf16, tag=f"x{q}", name=f"x{q}")
            ring = nc.scalar if q % 2 == 0 else nc.gpsimd
            if q == 0:
                # split the first tile so the k=0 chunk (and with it the very
                # first matmul) is ready half a DMA earlier
                ring.dma_start(t[:, 0:T + 4], xT[:, 0:T + 4])
                ring.dma_start(t[:, T + 4:W2], xT[:, T + 4:W2])
            else:
                ring.dma_start(t[:], xT[:, q * W2:(q + 1) * W2])
            xkt.append(t)

        def xs(k, c0):
            """rhs slice [c0, c0+S) of hidden k-chunk k (within a pair tile)."""
            base = (k % 2) * (T + 4) + c0
            return xkt[k // 2][:, base:base + S]
        cf = sb.tile([128, 48], f32, tag="cstf", name="cstf")
        nc.gpsimd.dma_start(cf[:], cstf[:])
        cb_t = cf[:, 0:IK]
        cpb_t = cf[:, IK:2 * IK]
        fcb_t = cf[:, 2 * IK:2 * IK + HK]
        sfc_t = cf[:, 2 * IK + HK:2 * IK + 2 * HK]
        cbt = sb.tile([128, 192], bf16, tag="cstb", name="cstb")
        nc.gpsimd.dma_start(cbt[:], cstb[:])
        ones_col = cbt[:, 0:1]
        ones128 = cbt[0:1, 0:128]
        carry = [cbt[:, 128 + i2 * 4:128 + i2 * 4 + 3] for i2 in range(IK)]

        res_t = [[None] * IK for _ in range(NSEG)]
        acc0 = [None] * NSEG   # f32 running sums over channel tiles
        acc1 = [None] * NSEG   # f32 running sums of squares

        # ---- in_proj + conv + gate, one conv group at a time, both segments
        for g in range(G):
            if g == 0:
                # fast start: four 256KB tiles (2KB rows keep the DMA queue at
                # full packet efficiency) so the first matmul waits ~1us
                w0s = []
                for kq in range(4):
                    wk = sb.tile([128, 1024], bf16, tag="w0s", bufs=4, name="w0s")
                    if kq == 0:
                        nc.sync.dma_start(wk[:, 0:512], win[:, 0:512])
                        nc.sync.dma_start(wk[:, 512:1024], win[:, 512:1024])
                    else:
                        nc.sync.dma_start(wk[:], win[:, kq * 1024:(kq + 1) * 1024])
                    w0s.append(wk)
                wsl = lambda k, c0: w0s[k // 2][:, (k % 2) * 512 + c0:(k % 2) * 512 + c0 + 128]
            else:
                wg = wblock(win, g)
                wsl = lambda k, c0: wg[:, k * 512 + c0:k * 512 + c0 + 128]
            cwt = sb.tile([128, 2048], bf16, tag="cw", bufs=2, name="cw")
            nc.scalar.dma_start(cwt[:], cw[:, g * 2048:(g + 1) * 2048])

            # k-outer over the four (p, m) accumulations: each arriving xk
            # tile immediately feeds 4 matmuls, so the cold-start x stream
            # never stalls the PE
            pq = [(p, m) for p in range(NSEG) for m in range(2)]
            hts = {}
            pms_h = {pm_: mm_ps() for pm_ in pq}
            for k in range(HK):
                for (p, m) in pq:
                    nc.tensor.matmul(pms_h[(p, m)][:], wsl(k, m * 128),
                                     xs(k, 4 + p * S),
                                     start=(k == 0), stop=(k == HK - 1))
            for (p, m) in pq:
                i2 = 2 * g + m
                ht = sb.tile([128, S + 4], bf16, tag="hT", bufs=6, name="hT")
                nc.vector.tensor_copy(ht[:, 4:S + 4], pms_h[(p, m)][:])
                if p == 0:
                    nc.vector.tensor_copy(ht[:, 1:4], carry[i2])
                else:
                    nc.vector.tensor_copy(ht[:, 1:4], hts[(0, m)][:, S + 1:S + 4])
                hts[(p, m)] = ht

            gss = {}
            pms_g = {pm_: mm_ps() for pm_ in pq}
            for k in range(HK):
                for (p, m) in pq:
                    nc.tensor.matmul(pms_g[(p, m)][:], wsl(k, 256 + m * 128),
                                     xs(k, 4 + p * S),
                                     start=(k == 0), stop=(k == HK - 1))
            for (p, m) in pq:
                gs = sb.tile([128, S], bf16, tag="gsc", bufs=8, name="gsc")
                nc.scalar.activation(gs[:], pms_g[(p, m)][:], AF.Silu)
                gss[(p, m)] = gs

            for p in range(NSEG):
                for m in range(2):
                    i2 = 2 * g + m
                    pc = mm_ps()
                    first = True
                    for cc in range(2):
                        for k in range(CK):
                            c0 = cc * 1024 + k * 256 + m * 128
                            nc.tensor.matmul(pc[:], cwt[:, c0:c0 + 128],
                                             hts[(p, cc)][:, k + 1:k + 1 + S],
                                             start=first,
                                             stop=(cc == 1 and k == CK - 1))
                            first = False
                    rs = sb.tile([128, S], bf16, tag=f"res{p}_{i2}",
                                 name=f"res{p}_{i2}")
                    # rs = (conv + bias) * silu(gate), fused on the DVE
                    nc.vector.scalar_tensor_tensor(rs[:], pc[:],
                                                   cb_t[:, i2:i2 + 1],
                                                   gss[(p, m)][:],
                                                   op0=ADD, op1=MUL)
                    res_t[p][i2] = rs
                    sq = sb.tile([128, S], bf16, tag="sq", bufs=4, name="sq")
                    nc.scalar.activation(sq[:], rs[:], AF.Square)
                    if i2 == 0:
                        a0 = sb.tile([128, S], f32, tag=f"acc0_{p}", name=f"acc0_{p}")
                        a1 = sb.tile([128, S], f32, tag=f"acc1_{p}", name=f"acc1_{p}")
                        nc.vector.tensor_copy(a0[:], rs[:])
                        nc.vector.tensor_copy(a1[:], sq[:])
                        acc0[p], acc1[p] = a0, a1
                    else:
                        nc.vector.tensor_tensor(acc0[p][:], acc0[p][:], rs[:], op=ADD)
                        nc.vector.tensor_tensor(acc1[p][:], acc1[p][:], sq[:], op=ADD)

        # ---- layernorm stats: one cross-partition ones-matmul per stat;
        # mean/rstd rows broadcast to all partitions on the (idle) GpSimd
        bcM_t, bcA_t = [None] * NSEG, [None] * NSEG
        for p in range(NSEG):
            a0b = sb.tile([128, S], bf16, tag="accb", bufs=2, name="accb")
            nc.vector.tensor_copy(a0b[:], acc0[p][:])
            a1b = sb.tile([128, S], bf16, tag="accb", bufs=2, name="accb")
            nc.vector.tensor_copy(a1b[:], acc1[p][:])
            pst0 = ps.tile([1, S], f32, tag="aux", bufs=1, name="auxps")
            nc.tensor.matmul(pst0[:], ones_col[:], a0b[:], start=True, stop=True)
            pst1 = ps.tile([1, S], f32, tag="aux", bufs=1, name="auxps")
            nc.tensor.matmul(pst1[:], ones_col[:], a1b[:], start=True, stop=True)
            mneg = sb.tile([1, S], bf16, tag=f"mneg{p}", name=f"mneg{p}")
            nc.vector.tensor_scalar(mneg[:], pst0[:], -1.0 / I, None, op0=MUL)
            msq = sb.tile([1, S], f32, tag="lnrow", bufs=2, name="msq")
            nc.scalar.activation(msq[:], pst0[:], AF.Square)
            nc.vector.tensor_scalar(msq[:], msq[:], 1.0 / I, None, op0=MUL)
            vrow = sb.tile([1, S], f32, tag="lnrow", bufs=2, name="vrow")
            nc.vector.tensor_tensor(vrow[:], pst1[:], msq[:], op=SUB)
            nc.vector.tensor_scalar(vrow[:], vrow[:], 1.0 / I, EPS, op0=MUL, op1=ADD)
            sd = sb.tile([1, S], f32, tag="lnrow", bufs=2, name="sd")
            nc.scalar.activation(sd[:], vrow[:], AF.Sqrt)
            arow = sb.tile([1, S], bf16, tag=f"arow{p}", name=f"arow{p}")
            nc.vector.reciprocal(arow[:], sd[:])                    # rstd
            bcA = sb.tile([128, S], bf16, tag=f"bcA{p}", name=f"bcA{p}")
            nc.gpsimd.partition_broadcast(bcA[:], arow[:])
            bcM = sb.tile([128, S], bf16, tag=f"bcM{p}", name=f"bcM{p}")
            nc.gpsimd.partition_broadcast(bcM[:], mneg[:])
            bcM_t[p], bcA_t[p] = bcM, bcA

        # ---- MLP c_fc (I -> H): m1 = silu((psum + S_fc*(-mean))*rstd + fcb);
        # the mean correction rides on the DVE (rank-1 matmuls are ~2.3x the
        # cost of a regular 512-col matmul on the PE)
        m1_t = [[None] * HK for _ in range(NSEG)]
        for jb in range(2):
            wbs = [wblock(fcw, jb * 2 + kb) for kb in range(2)]
            for p in range(NSEG):
                pms = [mm_ps() for _ in range(4)]
                for kb in range(2):
                    for j2 in range(4):
                        for kk in range(8):
                            nc.tensor.matmul(pms[j2][:],
                                             wbs[kb][:, kk * 512 + j2 * 128:kk * 512 + (j2 + 1) * 128],
                                             res_t[p][kb * 8 + kk][:],
                                             start=(kb == 0 and kk == 0),
                                             stop=(kb == 1 and kk == 7))
                for j2 in range(4):
                    j = jb * 4 + j2
                    t1 = sb.tile([128, S], bf16, tag="fctmp", bufs=4, name="fctmp")
                    nc.vector.scalar_tensor_tensor(t1[:], bcM_t[p][:],
                                                   sfc_t[:, j:j + 1],
                                                   pms[j2][:], op0=MUL, op1=ADD)
                    tmp = sb.tile([128, S], bf16, tag="fctmp", bufs=4, name="fctmp")
                    nc.vector.tensor_tensor(tmp[:], t1[:], bcA_t[p][:], op=MUL)
                    m1 = sb.tile([128, S], bf16, tag=f"m1_{p}_{j}", name=f"m1_{p}_{j}")
                    nc.scalar.activation(m1[:], tmp[:], AF.Silu,
                                         bias=fcb_t[:, j:j + 1])
                    m1_t[p][j] = m1

        # ---- MLP c_proj (H -> I) + bias + residual add (fused on DVE)
        oin_t = [[None] * IK for _ in range(NSEG)]
        for ib in range(4):
            wb = wblock(cpw, ib)
            for p in range(NSEG):
                for i2 in range(4):
                    i = ib * 4 + i2
                    pm = mm_ps()
                    for kk in range(HK):
                        nc.tensor.matmul(pm[:], wb[:, kk * 512 + i2 * 128:kk * 512 + (i2 + 1) * 128],
                                         m1_t[p][kk][:], start=(kk == 0),
                                         stop=(kk == HK - 1))
                    oi = sb.tile([128, S], bf16, tag=f"oin{p}_{i}",
                                 name=f"oin{p}_{i}")
                    nc.vector.scalar_tensor_tensor(oi[:], pm[:],
                                                   cpb_t[:, i:i + 1],
                                                   res_t[p][i][:],
                                                   op0=ADD, op1=ADD)
                    oin_t[p][i] = oi

        # ---- out_proj (I -> H); both segments land in one [128, 1024] output
        # tile per j-chunk so each DMA writes full 2KB rows, spread over rings
        for jb in range(2):
            wbs = [wblock(outw, jb * 2 + kb) for kb in range(2)]
            yos = [sb.tile([128, T], bf16, tag="yo", bufs=4, name="yo")
                   for _ in range(4)]
            for p in range(NSEG):
                t0 = p * S
                pms = [mm_ps() for _ in range(4)]
                for kb in range(2):
                    for j2 in range(4):
                        for kk in range(8):
                            nc.tensor.matmul(pms[j2][:],
                                             wbs[kb][:, kk * 512 + j2 * 128:kk * 512 + (j2 + 1) * 128],
                                             oin_t[p][kb * 8 + kk][:],
                                             start=(kb == 0 and kk == 0),
                                             stop=(kb == 1 and kk == 7))
                for j2 in range(4):
                    nc.vector.tensor_copy(yos[j2][:, t0:t0 + S], pms[j2][:])
            for j2 in range(4):
                j = jb * 4 + j2
                ring = (nc.scalar, nc.gpsimd, nc.sync, nc.scalar)[j2]
                ring.dma_start(yT[j * 128:(j + 1) * 128, :], yos[j2][:])

    nc.compile()
    return nc


def _pack(inputs):
    import ml_dtypes

    b16 = ml_dtypes.bfloat16
    f = lambda name: np.asarray(inputs[name], np.float32)
    hs = np.ascontiguousarray(f("hidden_states"))
    wT = np.ascontiguousarray(f("in_proj_w").T)                 # [H, 2I]
    winp = np.empty((H, 2 * I), np.float32)
    for g in range(G):
        winp[:, g * 512:g * 512 + 256] = wT[:, g * 256:(g + 1) * 256]
        winp[:, g * 512 + 256:(g + 1) * 512] = wT[:, I + g * 256:I + (g + 1) * 256]
    # block layouts: [128, nblocks*4096]; block b holds 8 consecutive lhsT
    # chunks [128, 512] so each phase-block is one contiguous 1MB DMA
    winb = np.ascontiguousarray(
        winp.reshape(HK, 128, G, 512).transpose(1, 2, 0, 3).reshape(128, HK * 2 * I)).astype(b16)
    # layernorm gamma/beta folded into c_fc (exact): silu((hn*g+b) @ W.T + c)
    # = silu(hn @ (W*g).T + (c + W @ b))
    fcw_eff = f("fc_w") * f("ln_g")[None, :]
    fcb_eff = f("fc_b") + f("fc_w") @ f("ln_b")
    sfc_col = np.ascontiguousarray(
        fcw_eff.sum(axis=1, dtype=np.float64).astype(np.float32).reshape(HK, 128).T)
    fcwb = np.ascontiguousarray(
        fcw_eff.T.reshape(2, 8, 128, 2, 512).transpose(2, 3, 0, 1, 4).reshape(128, I * H // 128)).astype(b16)
    cpwb = np.ascontiguousarray(
        f("cproj_w").T.reshape(8, 128, 4, 512).transpose(1, 2, 0, 3).reshape(128, H * I // 128)).astype(b16)
    outwb = np.ascontiguousarray(
        f("out_w").T.reshape(2, 8, 128, 2, 512).transpose(2, 3, 0, 1, 4).reshape(128, I * H // 128)).astype(b16)
    v = f("conv_w").reshape(G, 256, 2, 128, CK)                 # [g, j, cc, i, k]
    cwp = np.ascontiguousarray(v.transpose(3, 0, 2, 4, 1).reshape(128, G * 2 * CK * 256)).astype(b16)
    cstf = np.empty((128, 48), np.float32)
    cstf[:, 0:IK] = f("conv_b").reshape(IK, 128).T
    cstf[:, IK:2 * IK] = f("cproj_b").reshape(IK, 128).T
    cstf[:, 2 * IK:2 * IK + HK] = fcb_eff.reshape(HK, 128).T
    cstf[:, 2 * IK + HK:2 * IK + 2 * HK] = sfc_col
    shared = dict(
        win=winb, cw=cwp,
        fcw=fcwb,
        cpw=cpwb,
        outw=outwb,
        cstf=np.ascontiguousarray(cstf),
    )
    ipw_h = f("in_proj_w")[:I]                                  # [I, H]
    in_maps = []
    for c in range(NCORES):
        b, q = divmod(c, QC)
        own = hs[b, q * T:(q + 1) * T]                          # [T, H]
        prev = (np.zeros((3, H), np.float32) if q == 0
                else hs[b, q * T - 3:q * T])
        pad = np.zeros((1, H), np.float32)                      # data at col 4
        xf = np.concatenate([pad, prev, own], 0).T              # [H, T+4]
        # pair-tile layout [128, 8*(T+4)]: partition p holds k-chunk 2q rows
        # then k-chunk 2q+1 rows side by side (4KB DMA rows)
        xTc = np.ascontiguousarray(
            xf.reshape(4, 2, 128, T + 4).transpose(2, 0, 1, 3).reshape(128, 8 * (T + 4))).astype(b16)
        hh = np.zeros((IK, 128, 4), np.float32)
        hh[:, :, 0:3] = (ipw_h @ prev.T).reshape(IK, 128, 3)    # halo h columns
        hh = hh.transpose(1, 0, 2).reshape(128, IK * 4)
        cstb = np.empty((128, 192), np.float32)
        cstb[:, 0:128] = 1.0                                    # ones block
        cstb[:, 128:192] = hh
        in_maps.append(dict(xT=xTc, cstb=np.ascontiguousarray(cstb).astype(b16),
                            **shared))
    return in_maps


def _run(inputs, trace=False):
    from concourse.bass_utils import run_bass_kernel_spmd

    nc = _CACHE.get("nc")
    if nc is None:
        nc = _build()
        _CACHE["nc"] = nc
    in_maps = _pack(inputs)
    try:
        res = run_bass_kernel_spmd(nc, in_maps, core_ids=list(range(NCORES)),
                                   trace=trace)
    except Exception:
        # transient NRT_EXEC_UNIT_UNRECOVERABLE has been observed once after a
        # wedged prior run; one retry has always succeeded
        res = run_bass_kernel_spmd(nc, in_maps, core_ids=list(range(NCORES)),
                                   trace=trace)
    y = np.empty((B, L, H), np.float32)
    for c in range(NCORES):
        b, q = divmod(c, QC)
        y[b, q * T:(q + 1) * T, :] = res.results[c]["yT"].astype(np.float32).T
    return y, res


def kernel(**inputs) -> np.ndarray:
    y, _ = _run(inputs, trace=False)
    return y
